# revision 33
# baseline (speedup 1.0000x reference)
"""Trainium2 Bass kernel: 5-layer dense transformer prefill (Qwen3-style),
sharded across 8 NeuronCores.

Sharding: heads are tensor-parallel for QKV+attention (2 Q heads + 1 KV head
per core); the T=2048 sequence axis is sharded 256/core for o-proj, MLP,
residual stream and norms.  Cross-core exchange per layer: one AllGather of
the normed activations (1 MB) and one AllToAll of the attention output (2 MB).
No AllReduce anywhere.

All on-chip activations live transposed (feature dim on partitions, tokens on
the free axis) so every matmul contracts over the partition dim naturally.
"""
import numpy as np
from contextlib import ExitStack

import concourse.bass as bass
import concourse.bacc as bacc
import concourse.mybir as mybir
import concourse.tile as tile

DT = mybir.dt.float32
DTR = mybir.dt.float32r
AF = mybir.ActivationFunctionType
ALU = mybir.AluOpType

L, D, H, KV, HD, FF = 5, 1024, 16, 8, 128, 3072
T = 2048
NC = 8
HQ = H // NC          # 2 Q heads per core
TC = T // NC          # 256 own tokens
DK = D // 128         # 8 d-tiles
FK = FF // 128        # 24 f-tiles
EPS = 1e-6
ROPE_THETA = 1e6
SCALE = float(HD) ** -0.5
NEG = -1e30


def r(ap):
    return ap.bitcast(DTR)


def build_program():
    nc = bacc.Bacc("TRN2", target_bir_lowering=False, debug=False, num_devices=NC)

    # ---------------- I/O ----------------
    embt = nc.dram_tensor("embt", [DK, 128, TC], DT, kind="ExternalInput")
    qkvw = nc.dram_tensor("qkvw", [L, 128, DK * 512], DTR, kind="ExternalInput")
    owt = nc.dram_tensor("owt", [L, DK, 128, 16 * 128], DTR, kind="ExternalInput")
    gatewt = nc.dram_tensor("gatewt", [L, FK, 128, DK * 128], DTR, kind="ExternalInput")
    upwt = nc.dram_tensor("upwt", [L, FK, 128, DK * 128], DTR, kind="ExternalInput")
    downwt = nc.dram_tensor("downwt", [L, DK, 128, FK * 128], DTR, kind="ExternalInput")
    qnw = nc.dram_tensor("qnw", [L, 1, 128], DTR, kind="ExternalInput")
    knw = nc.dram_tensor("knw", [L, 1, 128], DTR, kind="ExternalInput")
    normw = nc.dram_tensor("normw", [1, DK * 128], DTR, kind="ExternalInput")
    cost_d = nc.dram_tensor("cost", [128, T], DT, kind="ExternalInput")
    sint_d = nc.dram_tensor("sint", [128, T], DT, kind="ExternalInput")
    maskb_d = nc.dram_tensor("maskb", [128, 896], DT, kind="ExternalInput")
    ident_d = nc.dram_tensor("ident", [128, 128], DTR, kind="ExternalInput")
    ones_d = nc.dram_tensor("ones", [128, 128], DTR, kind="ExternalInput")

    hs_out = nc.dram_tensor("hs_out", [TC, D], DT, kind="ExternalOutput")
    k_out = nc.dram_tensor("k_out", [L, T, HD], DT, kind="ExternalOutput")
    v_out = nc.dram_tensor("v_out", [L, T, HD], DT, kind="ExternalOutput")

    with tile.TileContext(nc) as tc, ExitStack() as top, \
         nc.allow_low_precision(reason="fp32r matmul operands (walrus rounding rule)"):
        const = top.enter_context(tc.tile_pool(name="const", bufs=1))
        cos_sb = const.tile([128, T], DT, name="cos_sb")
        sin_sb = const.tile([128, T], DT, name="sin_sb")
        mask_sb = const.tile([128, 896], DT, name="mask_sb")
        id_sb = const.tile([128, 128], DTR, name="id_sb")
        ones_sb = const.tile([128, 128], DTR, name="ones_sb")
        nw_sb = const.tile([1, DK * 128], DTR, name="nw_sb")
        nc.sync.dma_start(cos_sb[:], cost_d[:])
        nc.sync.dma_start(sin_sb[:], sint_d[:])
        nc.sync.dma_start(mask_sb[:], maskb_d[:])
        nc.sync.dma_start(id_sb[:], ident_d[:])
        nc.sync.dma_start(ones_sb[:], ones_d[:].bitcast(DTR))
        nc.sync.dma_start(nw_sb[:], normw[:].bitcast(DTR))
        eps_sb = const.tile([128, 1], DT, name="eps_sb")
        nc.gpsimd.memset(eps_sb[:], EPS)
        ones_col = ones_sb[:, 0:1]
        ones_row = ones_sb[0:1, :]

        hp = top.enter_context(tc.tile_pool(name="hp", bufs=1))
        h_sb = hp.tile([128, DK * TC], DT, name="h_sb")     # h^T: [p, d*TC+t]
        nc.sync.dma_start(h_sb[:].rearrange("p (k t) -> p k t", k=DK),
                          embt[:].rearrange("k p t -> p k t"))

        dram = top.enter_context(tc.tile_pool(name="dram", bufs=2, space="DRAM"))

        def rms_rinv(pool, ps_pool, src_sb, ktiles, mean_n, tag):
            """src_sb: (128, ktiles*TC) feature-major tiles -> rinv (1, TC) SBUF."""
            ssq_ps = ps_pool.tile([1, TC], DT, tag="ssq", name=f"ssq_{tag}", bufs=1)
            for d in range(ktiles):
                sq = pool.tile([128, TC], DTR, tag="sq", name=f"sq_{tag}{d}")
                nc.scalar.activation(sq[:], src_sb[:, d * TC:(d + 1) * TC], AF.Square)
                nc.tensor.matmul(ssq_ps[:], r(ones_col), r(sq[:]),
                                 start=(d == 0), stop=(d == ktiles - 1))
            sqv = pool.tile([1, TC], DT, tag="sqv", name=f"sqv_{tag}")
            nc.scalar.activation(sqv[:], ssq_ps[:], AF.Sqrt, scale=1.0 / mean_n,
                                 bias=eps_sb[0:1, :])
            rinv = pool.tile([1, TC], DTR, tag="rinv", name=f"rinv_{tag}")
            nc.vector.reciprocal(rinv[:], sqv[:])
            return rinv

        for l in range(L):
            # ======== Phase A: x^T = rms(h^T) on own cols, AllGather ========
            ag_in = dram.tile([D, TC], DTR, tag="ag_in", name=f"ag_in{l}")
            ag_out = dram.tile([NC * D, TC], DTR, tag="ag_out", name=f"ag_out{l}",
                               addr_space="Shared")
            with tc.tile_pool(name=f"pA{l}", bufs=2) as pa, \
                 tc.tile_pool(name=f"psA{l}", bufs=2, space="PSUM") as psa:
                rinv = rms_rinv(pa, psa, h_sb, DK, float(D), f"x{l}")
                bc_ps = psa.tile([128, TC], DT, tag="bc", name=f"bcx{l}", bufs=1)
                nc.tensor.matmul(bc_ps[:], r(ones_row), r(rinv[:]),
                                 start=True, stop=True)
                for d in range(DK):
                    xt = pa.tile([128, TC], DTR, tag="xt", name=f"xt{l}{d}")
                    nc.vector.tensor_tensor(
                        xt[:], h_sb[:, d * TC:(d + 1) * TC], bc_ps[:], ALU.mult)
                    nc.sync.dma_start(ag_in[128 * d:128 * (d + 1), :], xt[:])
            nc.gpsimd.collective_compute(
                "AllGather", ALU.bypass, replica_groups=[list(range(NC))],
                ins=[ag_in.opt()], outs=[ag_out.opt()])

            # xg view: [k][p, r, t] = x^T[128k+p, TC*r+t]
            xg_view = ag_out.rearrange("(r k p) t -> k p r t", r=NC, k=DK)

            # ======== Phase B: QKV + qk-norm + rope + transposes ========
            a2a_in = dram.tile([T, TC], DTR, tag="a2a_in", name=f"a2a_in{l}")
            a2a_out = dram.tile([T, TC], DTR, tag="a2a_out", name=f"a2a_out{l}")
            with tc.tile_pool(name=f"pB{l}", bufs=1) as pb, \
                 tc.tile_pool(name=f"pBs{l}", bufs=2) as pbs, \
                 tc.tile_pool(name=f"pBx{l}", bufs=3) as pbx:
                psb_ctx = ExitStack()
                psb = psb_ctx.enter_context(
                    tc.tile_pool(name=f"psB{l}", bufs=2, space="PSUM"))
                wq_sb = pb.tile([128, DK * 512], DTR, name=f"wq_sb{l}")
                nc.sync.dma_start(wq_sb[:], qkvw[l].bitcast(DTR))
                qn_sb = pbs.tile([1, 128], DTR, tag="qn", name=f"qn_sb{l}")
                kn_sb = pbs.tile([1, 128], DTR, tag="kn", name=f"kn_sb{l}")
                nc.sync.dma_start(qn_sb[:], qnw[l].bitcast(DTR))
                nc.sync.dma_start(kn_sb[:], knw[l].bitcast(DTR))
                qT = [pb.tile([128, T], DTR, name=f"qT{l}{m}") for m in range(HQ)]
                kT = pb.tile([128, T], DTR, name=f"kT{l}")
                vT = pb.tile([128, T], DTR, name=f"vT{l}")
                vnat = pb.tile([128, T], DTR, name=f"vnat{l}")

                for c in range(4):
                    xg = [pbx.tile([128, 512], DTR, tag=f"xg{k}", bufs=2,
                                   name=f"xg{l}{c}{k}") for k in range(DK)]
                    for k in range(DK):
                        nc.sync.dma_start(
                            xg[k][:].rearrange("p (r t) -> p r t", r=2),
                            xg_view[k][:, 2 * c:2 * c + 2, :].bitcast(DTR))
                    for m in range(4):
                        acc = psb.tile([128, 512], DT, tag="acc", name=f"acc{l}{c}{m}")
                        for k in range(DK):
                            nc.tensor.matmul(
                                acc[:],
                                r(wq_sb[:, k * 512 + m * 128: k * 512 + (m + 1) * 128]),
                                r(xg[k][:]),
                                start=(k == 0), stop=(k == DK - 1))
                        cs = slice(512 * c, 512 * (c + 1))
                        if m == 3:
                            nc.scalar.activation(vT[:, cs], acc[:], AF.Copy)
                            continue
                        # qk rms-norm over the 128 partition dims
                        sq = pbs.tile([128, 512], DTR, tag="sqb", name=f"sqb{l}{c}{m}")
                        nc.scalar.activation(sq[:], acc[:], AF.Square)
                        ssq = psb.tile([1, 512], DT, tag="ssqb", name=f"ssqb{l}{c}{m}",
                                       bufs=1)
                        nc.tensor.matmul(ssq[:], r(ones_col), r(sq[:]),
                                         start=True, stop=True)
                        sqv = pbs.tile([1, 512], DT, tag="sqvb", name=f"sqvb{l}{c}{m}")
                        nc.scalar.activation(sqv[:], ssq[:], AF.Sqrt,
                                             scale=1.0 / HD, bias=eps_sb[0:1, :])
                        rin = pbs.tile([1, 512], DTR, tag="rinb", name=f"rinb{l}{c}{m}")
                        nc.vector.reciprocal(rin[:], sqv[:])
                        bw_ps = psb.tile([128, 512], DT, tag="bc", name=f"bw{l}{c}{m}",
                                         bufs=1)
                        w_sb = qn_sb if m < 2 else kn_sb
                        nc.tensor.matmul(bw_ps[:], r(w_sb[:]), r(rin[:]),
                                         start=True, stop=True)
                        bw_sb = pbs.tile([128, 512], DT, tag="bwsb",
                                         name=f"bwsb{l}{c}{m}")
                        nc.scalar.activation(bw_sb[:], bw_ps[:], AF.Copy)
                        qn_t = pbs.tile([128, 512], DT, tag="qnt", name=f"qnt{l}{c}{m}")
                        nc.vector.tensor_tensor(qn_t[:], acc[:], bw_sb[:], ALU.mult)
                        # rope: out = q*cos + rot_half(q)*sin_signed.
                        # qswap = q rotated by 64 partitions (SBUF DMA moves
                        # across partitions; DVE cannot).  sin_sb carries the
                        # sign: rows 0-63 = -sin, 64-127 = +sin.
                        dst = (qT[m] if m < HQ else kT)[:, cs]
                        qsw = pbs.tile([128, 512], DT, tag="qsw", name=f"qsw{l}{c}{m}")
                        nc.sync.dma_start(qsw[0:64, :], qn_t[64:128, :])
                        nc.sync.dma_start(qsw[64:128, :], qn_t[0:64, :])
                        nc.vector.tensor_tensor(dst, qn_t[:], cos_sb[:, cs], ALU.mult)
                        ru = pbs.tile([128, 512], DT, tag="ru", name=f"ru{l}{c}{m}")
                        nc.vector.tensor_tensor(ru[:], qsw[:], sin_sb[:, cs], ALU.mult)
                        nc.vector.tensor_tensor(dst, dst, ru[:], ALU.add)

                # transposes: v -> natural (also PV lhsT), k -> natural for output
                for i in range(16):
                    isl = slice(128 * i, 128 * (i + 1))
                    tpv = psb.tile([128, 128], DTR, tag="tp", name=f"tpv{l}{i}")
                    nc.tensor.transpose(tpv[:], vT[:, isl], id_sb[:])
                    nc.scalar.activation(vnat[:, isl], tpv[:], AF.Copy)
                    nc.sync.dma_start(v_out[l][isl, :].bitcast(DTR), vnat[:, isl])
                    tpk = psb.tile([128, 128], DTR, tag="tpk", name=f"tpk{l}{i}", bufs=1)
                    nc.tensor.transpose(tpk[:], kT[:, isl], id_sb[:])
                    ktr = pbs.tile([128, 128], DT, tag="ktr", name=f"ktr{l}{i}")
                    nc.vector.tensor_copy(ktr[:], tpk[:])
                    nc.sync.dma_start(k_out[l][isl, :], ktr[:])
                psb_ctx.close()

                # ======== Phase C: attention (causal) ========
                with tc.tile_pool(name=f"pC{l}", bufs=3) as pc, \
                     tc.tile_pool(name=f"psC{l}", bufs=2, space="PSUM") as psc:
                    for hq in range(HQ):
                        for c in range(4):
                            cs = slice(512 * c, 512 * (c + 1))
                            pv = psc.tile([128, 512], DT, tag="pv",
                                          name=f"pv{l}{hq}{c}")
                            rs = psc.tile([1, 512], DT, tag="rs", name=f"rs{l}{hq}{c}")
                            nlast = 4 * c + 3
                            for i in range(4 * c + 4):
                                s_ps = psc.tile([128, 512], DT, tag="s",
                                                name=f"s{l}{hq}{c}{i}")
                                nc.tensor.matmul(
                                    s_ps[:], r(kT[:, 128 * i:128 * (i + 1)]),
                                    r(qT[hq][:, cs]), start=True, stop=True)
                                if i >= 4 * c:
                                    off = 128 * i - 512 * c
                                    nc.vector.tensor_tensor(
                                        s_ps[:], s_ps[:],
                                        mask_sb[:, 384 - off: 896 - off], ALU.add)
                                p_sb = pc.tile([128, 512], DTR, tag="p",
                                               name=f"p{l}{hq}{c}{i}")
                                nc.scalar.activation(p_sb[:], s_ps[:], AF.Exp,
                                                     scale=SCALE)
                                nc.tensor.matmul(
                                    pv[:], r(vnat[:, 128 * i:128 * (i + 1)]),
                                    r(p_sb[:]), start=(i == 0), stop=(i == nlast))
                                nc.tensor.matmul(
                                    rs[:], r(ones_col), r(p_sb[:]),
                                    start=(i == 0), stop=(i == nlast))
                            rr = pc.tile([1, 512], DTR, tag="rr", name=f"rr{l}{hq}{c}")
                            nc.vector.reciprocal(rr[:], rs[:])
                            bca = psc.tile([128, 512], DT, tag="bca",
                                           name=f"bca{l}{hq}{c}", bufs=1)
                            nc.tensor.matmul(bca[:], r(ones_row), r(rr[:]),
                                             start=True, stop=True)
                            bcs = pc.tile([128, 512], DT, tag="bcs",
                                          name=f"bcs{l}{hq}{c}")
                            nc.scalar.activation(bcs[:], bca[:], AF.Copy)
                            ao = pc.tile([128, 512], DTR, tag="ao", name=f"ao{l}{hq}{c}")
                            nc.vector.tensor_tensor(ao[:], pv[:], bcs[:], ALU.mult)
                            for sub in range(2):
                                r0 = 512 * c + 256 * sub + 128 * hq
                                nc.sync.dma_start(
                                    a2a_in[r0:r0 + 128, :],
                                    ao[:, 256 * sub:256 * (sub + 1)])

            nc.gpsimd.collective_compute(
                "AllToAll", ALU.bypass, replica_groups=[list(range(NC))],
                ins=[a2a_in.opt()], outs=[a2a_out.opt()])

            # ======== Phase D: o-proj + residual ========
            with tc.tile_pool(name=f"pD{l}", bufs=1) as pd, \
                 tc.tile_pool(name=f"pDw{l}", bufs=2) as pdw, \
                 tc.tile_pool(name=f"psD{l}", bufs=2, space="PSUM") as psd:
                ax_sb = pd.tile([128, 16 * TC], DTR, name=f"ax_sb{l}")
                nc.sync.dma_start(ax_sb[:].rearrange("p (k t) -> p k t", k=16),
                                  a2a_out.rearrange("(k p) t -> p k t",
                                                    k=16).bitcast(DTR))
                for d in range(DK):
                    ow_sb = pdw.tile([128, 16 * 128], DTR, tag="ow", name=f"ow{l}{d}")
                    nc.sync.dma_start(ow_sb[:], owt[l, d].bitcast(DTR))
                    op_ps = psd.tile([128, TC], DT, tag="op", name=f"op{l}{d}")
                    for kt in range(16):
                        nc.tensor.matmul(
                            op_ps[:], r(ow_sb[:, kt * 128:(kt + 1) * 128]),
                            r(ax_sb[:, kt * TC:(kt + 1) * TC]),
                            start=(kt == 0), stop=(kt == 15))
                    dsl = slice(d * TC, (d + 1) * TC)
                    nc.vector.tensor_tensor(h_sb[:, dsl], h_sb[:, dsl], op_ps[:],
                                            ALU.add)

            # ======== Phase E: MLP ========
            with tc.tile_pool(name=f"pE{l}", bufs=1) as pe, \
                 tc.tile_pool(name=f"pEs{l}", bufs=2) as pes, \
                 tc.tile_pool(name=f"pEw{l}", bufs=3) as pew, \
                 tc.tile_pool(name=f"psE{l}", bufs=2, space="PSUM") as pse:
                rinv2 = rms_rinv(pes, pse, h_sb, DK, float(D), f"y{l}")
                bcy = pse.tile([128, TC], DT, tag="bc", name=f"bcy{l}", bufs=1)
                nc.tensor.matmul(bcy[:], r(ones_row), r(rinv2[:]), start=True,
                                 stop=True)
                y_sb = pe.tile([128, DK * TC], DTR, name=f"y_sb{l}")
                for d in range(DK):
                    dsl = slice(d * TC, (d + 1) * TC)
                    nc.vector.tensor_tensor(y_sb[:, dsl], h_sb[:, dsl], bcy[:],
                                            ALU.mult)
                gu_sb = pe.tile([128, FK * TC], DTR, name=f"gu_sb{l}")
                for f in range(FK):
                    gw = pew.tile([128, DK * 128], DTR, tag="gw", name=f"gw{l}{f}")
                    uw = pew.tile([128, DK * 128], DTR, tag="uw", name=f"uw{l}{f}")
                    nc.sync.dma_start(gw[:], gatewt[l, f].bitcast(DTR))
                    nc.sync.dma_start(uw[:], upwt[l, f].bitcast(DTR))
                    g_ps = pse.tile([128, TC], DT, tag="g", name=f"g{l}{f}")
                    u_ps = pse.tile([128, TC], DT, tag="u", name=f"u{l}{f}")
                    for k in range(DK):
                        ksl = slice(k * 128, (k + 1) * 128)
                        ysl = slice(k * TC, (k + 1) * TC)
                        nc.tensor.matmul(g_ps[:], r(gw[:, ksl]), r(y_sb[:, ysl]),
                                         start=(k == 0), stop=(k == DK - 1))
                        nc.tensor.matmul(u_ps[:], r(uw[:, ksl]), r(y_sb[:, ysl]),
                                         start=(k == 0), stop=(k == DK - 1))
                    sg = pes.tile([128, TC], DT, tag="sg", name=f"sg{l}{f}")
                    nc.scalar.activation(sg[:], g_ps[:], AF.Sigmoid)
                    gs = pes.tile([128, TC], DT, tag="gs", name=f"gs{l}{f}")
                    nc.vector.tensor_tensor(gs[:], sg[:], g_ps[:], ALU.mult)
                    fsl = slice(f * TC, (f + 1) * TC)
                    nc.vector.tensor_tensor(gu_sb[:, fsl], gs[:], u_ps[:], ALU.mult)
                for d in range(DK):
                    dw = pew.tile([128, FK * 128], DTR, tag="dw", name=f"dw{l}{d}",
                                  bufs=2)
                    nc.sync.dma_start(dw[:], downwt[l, d].bitcast(DTR))
                    dp = pse.tile([128, TC], DT, tag="dp", name=f"dp{l}{d}")
                    for f in range(FK):
                        nc.tensor.matmul(
                            dp[:], r(dw[:, f * 128:(f + 1) * 128]),
                            r(gu_sb[:, f * TC:(f + 1) * TC]),
                            start=(f == 0), stop=(f == FK - 1))
                    dsl = slice(d * TC, (d + 1) * TC)
                    nc.vector.tensor_tensor(h_sb[:, dsl], h_sb[:, dsl], dp[:],
                                            ALU.add)

        # ======== Final norm + transpose + store ========
        with tc.tile_pool(name="pF", bufs=2) as pf, \
             tc.tile_pool(name="psF", bufs=2, space="PSUM") as psf:
            rinvf = rms_rinv(pf, psf, h_sb, DK, float(D), "f")
            for d in range(DK):
                bcf = psf.tile([128, TC], DT, tag="bc", name=f"bcf{d}", bufs=1)
                nc.tensor.matmul(bcf[:], r(nw_sb[:, d * 128:(d + 1) * 128]),
                                 r(rinvf[:]), start=True, stop=True)
                hs_sb = pf.tile([128, TC], DTR, tag="hs", name=f"hs{d}")
                nc.vector.tensor_tensor(hs_sb[:], h_sb[:, d * TC:(d + 1) * TC],
                                        bcf[:], ALU.mult)
                for s in range(2):
                    tpf = psf.tile([128, 128], DTR, tag="tp", name=f"tpf{d}{s}")
                    nc.tensor.transpose(tpf[:], hs_sb[:, 128 * s:128 * (s + 1)],
                                        id_sb[:])
                    tfs = pf.tile([128, 128], DT, tag="tfs", name=f"tfs{d}{s}")
                    nc.scalar.activation(tfs[:], tpf[:], AF.Copy)
                    nc.sync.dma_start(
                        hs_out[128 * s:128 * (s + 1), 128 * d:128 * (d + 1)],
                        tfs[:])
    nc.compile()
    return nc


# ---------------- host side ----------------

def _rope_tables():
    inv_freq = 1.0 / (ROPE_THETA ** (np.arange(0, HD, 2, dtype=np.float64) / HD))
    pos = np.arange(T, dtype=np.float64)
    freqs = pos[:, None] * inv_freq[None, :]
    cos_h = np.cos(freqs).T.astype(np.float32)           # (64, T)
    sin_h = np.sin(freqs).T.astype(np.float32)
    cos128 = np.concatenate([cos_h, cos_h], axis=0)      # duplicated halves
    sin128 = np.concatenate([-sin_h, sin_h], axis=0)     # sign folded in
    return np.ascontiguousarray(cos128), np.ascontiguousarray(sin128)


def _mask_buffer():
    u = np.arange(896)[None, :]
    p = np.arange(128)[:, None]
    return np.where(u >= p + 384, 0.0, NEG).astype(np.float32)


def host_prep(inputs):
    f = lambda a: np.ascontiguousarray(np.asarray(a, np.float32))
    emb = f(inputs["inputs_embeds"])[0]
    ln1, ln2 = f(inputs["ln1_w"]), f(inputs["ln2_w"])
    qw, kw, vw = f(inputs["q_w"]), f(inputs["k_w"]), f(inputs["v_w"])
    qn, kn = f(inputs["qn_w"]), f(inputs["kn_w"])
    ow = f(inputs["o_w"])
    gw, uw, dw = f(inputs["gate_w"]), f(inputs["up_w"]), f(inputs["down_w"])
    nw = f(inputs["norm_w"])

    cosT, sinT = _rope_tables()
    maskB = _mask_buffer()
    ident = np.eye(128, dtype=np.float32)
    ones = np.ones((128, 128), np.float32)

    owT = ow.transpose(0, 2, 1)
    owt = np.ascontiguousarray(
        owT.reshape(L, 16, 128, DK, 128).transpose(0, 3, 2, 1, 4)
           .reshape(L, DK, 128, 16 * 128))

    def guw_tile(w):
        wf = (w * ln2[:, None, :]).transpose(0, 2, 1)
        return np.ascontiguousarray(
            wf.reshape(L, DK, 128, FK, 128).transpose(0, 3, 2, 1, 4)
              .reshape(L, FK, 128, DK * 128))
    gatewt, upwt = guw_tile(gw), guw_tile(uw)

    dwT = dw.transpose(0, 2, 1)
    downwt = np.ascontiguousarray(
        dwT.reshape(L, FK, 128, DK, 128).transpose(0, 3, 2, 1, 4)
           .reshape(L, DK, 128, FK * 128))

    shared = dict(owt=owt, gatewt=gatewt, upwt=upwt, downwt=downwt,
                  qnw=np.ascontiguousarray(qn[:, None, :]),
                  knw=np.ascontiguousarray(kn[:, None, :]),
                  normw=np.ascontiguousarray(nw[None, :]),
                  cost=cosT, sint=sinT, maskb=maskB, ident=ident, ones=ones)

    in_maps = []
    for c in range(NC):
        qs = qw[:, 256 * c:256 * (c + 1), :] * ln1[:, None, :]
        ks = kw[:, 128 * c:128 * (c + 1), :] * ln1[:, None, :]
        vs = vw[:, 128 * c:128 * (c + 1), :] * ln1[:, None, :]
        wT = np.concatenate([qs, ks, vs], axis=1).transpose(0, 2, 1)  # (L, D, 512)
        qkvw = np.ascontiguousarray(
            wT.reshape(L, DK, 128, 512).transpose(0, 2, 1, 3)
              .reshape(L, 128, DK * 512))
        embt = np.ascontiguousarray(
            emb.T[:, TC * c:TC * (c + 1)].reshape(DK, 128, TC))
        in_maps.append(dict(embt=embt, qkvw=qkvw, **shared))
    return in_maps


def assemble(results):
    hs = np.concatenate([res["hs_out"] for res in results], axis=0)[None]
    keys = np.stack([np.stack([res["k_out"][l] for res in results])
                     for l in range(L)])[:, None]
    vals = np.stack([np.stack([res["v_out"][l] for res in results])
                     for l in range(L)])[:, None]
    return (np.ascontiguousarray(hs, dtype=np.float32),
            np.ascontiguousarray(keys, dtype=np.float32),
            np.ascontiguousarray(vals, dtype=np.float32))


_CACHE = {}


def kernel(**inputs):
    from concourse.bass_utils import run_bass_kernel_spmd
    if "nc" not in _CACHE:
        _CACHE["nc"] = build_program()
    in_maps = host_prep(inputs)
    res = run_bass_kernel_spmd(_CACHE["nc"], in_maps, core_ids=list(range(NC)))
    return assemble(res.results)


# revision 34
# speedup vs baseline: 1.2111x; 1.2111x over previous
"""Trainium2 Bass kernel: 5-layer dense transformer prefill (Qwen3-style),
sharded across 8 NeuronCores.

Sharding: heads are tensor-parallel for QKV+attention (2 Q heads + 1 KV head
per core); the T=2048 sequence axis is sharded 256/core for o-proj, MLP,
residual stream and norms.  Cross-core exchange per layer: one AllGather of
the normed activations (1 MB) and one AllToAll of the attention output (2 MB).
No AllReduce anywhere.

All on-chip activations live transposed (feature dim on partitions, tokens on
the free axis) so every matmul contracts over the partition dim naturally.
"""
import numpy as np
from contextlib import ExitStack

import concourse.bass as bass
import concourse.bacc as bacc
import concourse.mybir as mybir
import concourse.tile as tile

DT = mybir.dt.float32
DTR = mybir.dt.float32r
AF = mybir.ActivationFunctionType
ALU = mybir.AluOpType

L, D, H, KV, HD, FF = 5, 1024, 16, 8, 128, 3072
T = 2048
NC = 8
HQ = H // NC          # 2 Q heads per core
TC = T // NC          # 256 own tokens
DK = D // 128         # 8 d-tiles
FK = FF // 128        # 24 f-tiles
EPS = 1e-6
ROPE_THETA = 1e6
SCALE = float(HD) ** -0.5
NEG = -1e30


def r(ap):
    return ap.bitcast(DTR)


def build_program():
    nc = bacc.Bacc("TRN2", target_bir_lowering=False, debug=False, num_devices=NC)

    # ---------------- I/O ----------------
    embt = nc.dram_tensor("embt", [DK, 128, TC], DT, kind="ExternalInput")
    qkvw = nc.dram_tensor("qkvw", [L, 128, DK * 512], DTR, kind="ExternalInput")
    owt = nc.dram_tensor("owt", [L, DK, 128, 16 * 128], DTR, kind="ExternalInput")
    gatewt = nc.dram_tensor("gatewt", [L, FK, 128, DK * 128], DTR, kind="ExternalInput")
    upwt = nc.dram_tensor("upwt", [L, FK, 128, DK * 128], DTR, kind="ExternalInput")
    downwt = nc.dram_tensor("downwt", [L, DK, 128, FK * 128], DTR, kind="ExternalInput")
    qnw = nc.dram_tensor("qnw", [L, 1, 128], DTR, kind="ExternalInput")
    knw = nc.dram_tensor("knw", [L, 1, 128], DTR, kind="ExternalInput")
    normw = nc.dram_tensor("normw", [1, DK * 128], DTR, kind="ExternalInput")
    cost_d = nc.dram_tensor("cost", [128, T], DT, kind="ExternalInput")
    sint_d = nc.dram_tensor("sint", [128, T], DT, kind="ExternalInput")
    maskb_d = nc.dram_tensor("maskb", [128, 896], DT, kind="ExternalInput")
    ident_d = nc.dram_tensor("ident", [128, 128], DTR, kind="ExternalInput")
    ones_d = nc.dram_tensor("ones", [128, 128], DTR, kind="ExternalInput")

    hs_out = nc.dram_tensor("hs_out", [TC, D], DT, kind="ExternalOutput")
    k_out = nc.dram_tensor("k_out", [L, T, HD], DT, kind="ExternalOutput")
    v_out = nc.dram_tensor("v_out", [L, T, HD], DT, kind="ExternalOutput")

    with tile.TileContext(nc) as tc, ExitStack() as top, \
         nc.allow_low_precision(reason="fp32r matmul operands (walrus rounding rule)"):
        const = top.enter_context(tc.tile_pool(name="const", bufs=1))
        cos_sb = const.tile([128, T], DT, name="cos_sb")
        sin_sb = const.tile([128, T], DT, name="sin_sb")
        mask_sb = const.tile([128, 896], DT, name="mask_sb")
        id_sb = const.tile([128, 128], DTR, name="id_sb")
        ones_sb = const.tile([128, 128], DTR, name="ones_sb")
        nw_sb = const.tile([1, DK * 128], DTR, name="nw_sb")
        nc.sync.dma_start(cos_sb[:], cost_d[:])
        nc.sync.dma_start(sin_sb[:], sint_d[:])
        nc.sync.dma_start(mask_sb[:], maskb_d[:])
        nc.sync.dma_start(id_sb[:], ident_d[:])
        nc.sync.dma_start(ones_sb[:], ones_d[:].bitcast(DTR))
        nc.sync.dma_start(nw_sb[:], normw[:].bitcast(DTR))
        eps_sb = const.tile([128, 1], DT, name="eps_sb")
        nc.gpsimd.memset(eps_sb[:], EPS)
        ones_col = ones_sb[:, 0:1]
        ones_row = ones_sb[0:1, :]

        hp = top.enter_context(tc.tile_pool(name="hp", bufs=1))
        h_sb = hp.tile([128, DK * TC], DT, name="h_sb")     # h^T: [p, d*TC+t]
        nc.sync.dma_start(h_sb[:].rearrange("p (k t) -> p k t", k=DK),
                          embt[:].rearrange("k p t -> p k t"))

        dram = top.enter_context(tc.tile_pool(name="dram", bufs=2, space="DRAM"))

        def rms_rinv(pool, ps_pool, src_sb, ktiles, mean_n, tag):
            """src_sb: (128, ktiles*TC) feature-major tiles -> rinv (1, TC) SBUF."""
            ssq_ps = ps_pool.tile([1, TC], DT, tag="ssq", name=f"ssq_{tag}", bufs=1)
            for d in range(ktiles):
                sq = pool.tile([128, TC], DTR, tag="sq", name=f"sq_{tag}{d}")
                nc.scalar.activation(sq[:], src_sb[:, d * TC:(d + 1) * TC], AF.Square)
                nc.tensor.matmul(ssq_ps[:], r(ones_col), r(sq[:]),
                                 start=(d == 0), stop=(d == ktiles - 1))
            sqv = pool.tile([1, TC], DT, tag="sqv", name=f"sqv_{tag}")
            nc.scalar.activation(sqv[:], ssq_ps[:], AF.Sqrt, scale=1.0 / mean_n,
                                 bias=eps_sb[0:1, :])
            rinv = pool.tile([1, TC], DT, tag="rinv", name=f"rinv_{tag}")
            nc.vector.reciprocal_approx_fast(rinv[:], sqv[:])
            return rinv

        kvp = top.enter_context(tc.tile_pool(name="kvp", bufs=2))
        deferred_k = []   # [(layer, kT tile)] -> transposed + stored later

        def emit_k_out(pool, ps_pool):
            while deferred_k:
                lk, kTp = deferred_k.pop()
                for i in range(16):
                    isl = slice(128 * i, 128 * (i + 1))
                    tpk = ps_pool.tile([128, 128], DTR, tag="tpk",
                                       name=f"tpk{lk}{i}", bufs=2)
                    nc.tensor.transpose(tpk[:], kTp[:, isl], id_sb[:])
                    ktr = pool.tile([128, 128], DT, tag="ktr", name=f"ktr{lk}{i}")
                    nc.vector.tensor_copy(ktr[:], tpk[:])
                    nc.sync.dma_start(k_out[lk][isl, :], ktr[:])

        for l in range(L):
            # ======== Phase A: x^T = rms(h^T) on own cols, AllGather ========
            ag_in = dram.tile([D, TC], DTR, tag="ag_in", name=f"ag_in{l}")
            ag_out = dram.tile([NC * D, TC], DTR, tag="ag_out", name=f"ag_out{l}",
                               addr_space="Shared")
            with tc.tile_pool(name=f"pA{l}", bufs=2) as pa, \
                 tc.tile_pool(name=f"psA{l}", bufs=2, space="PSUM") as psa:
                rinv = rms_rinv(pa, psa, h_sb, DK, float(D), f"x{l}")
                bwx = pa.tile([128, TC], DT, tag="bwx", name=f"bwx{l}")
                nc.gpsimd.partition_broadcast(bwx[:], rinv[:])
                for d in range(DK):
                    xt = pa.tile([128, TC], DTR, tag="xt", name=f"xt{l}{d}")
                    nc.vector.tensor_tensor(
                        xt[:], h_sb[:, d * TC:(d + 1) * TC], bwx[:], ALU.mult)
                    nc.sync.dma_start(ag_in[128 * d:128 * (d + 1), :], xt[:])
                nc.gpsimd.collective_compute(
                    "AllGather", ALU.bypass, replica_groups=[list(range(NC))],
                    ins=[ag_in.opt()], outs=[ag_out.opt()])
                # fill the AllGather window with last layer's k transposes
                emit_k_out(pa, psa)

            # xg view: [k][p, r, t] = x^T[128k+p, TC*r+t]
            xg_view = ag_out.rearrange("(r k p) t -> k p r t", r=NC, k=DK)

            # ======== Phase B: QKV + qk-norm + rope + transposes ========
            a2a_in = [dram.tile([NC * 128, TC], DTR, tag=f"a2a_in{h}",
                                name=f"a2a_in{l}{h}") for h in range(HQ)]
            a2a_out = [dram.tile([NC * 128, TC], DTR, tag=f"a2a_out{h}",
                                 name=f"a2a_out{l}{h}") for h in range(HQ)]
            with tc.tile_pool(name=f"pB{l}", bufs=1) as pb, \
                 tc.tile_pool(name=f"pBs{l}", bufs=2) as pbs, \
                 tc.tile_pool(name=f"pBx{l}", bufs=3) as pbx:
                psb_ctx = ExitStack()
                psb = psb_ctx.enter_context(
                    tc.tile_pool(name=f"psB{l}", bufs=2, space="PSUM"))
                wq_sb = pb.tile([128, DK * 512], DTR, name=f"wq_sb{l}")
                nc.sync.dma_start(wq_sb[:], qkvw[l].bitcast(DTR))
                qn_sb = pbs.tile([1, 128], DTR, tag="qn", name=f"qn_sb{l}")
                kn_sb = pbs.tile([1, 128], DTR, tag="kn", name=f"kn_sb{l}")
                nc.sync.dma_start(qn_sb[:], qnw[l].bitcast(DTR))
                nc.sync.dma_start(kn_sb[:], knw[l].bitcast(DTR))
                qT = [pb.tile([128, T], DTR, name=f"qT{l}{m}") for m in range(HQ)]
                kT = kvp.tile([128, T], DTR, tag="kT", name=f"kT{l}")
                vT = pb.tile([128, T], DTR, name=f"vT{l}")
                vnat = pb.tile([128, T], DTR, name=f"vnat{l}")

                for c in range(4):
                    xg = [pbx.tile([128, 512], DTR, tag=f"xg{k}", bufs=2,
                                   name=f"xg{l}{c}{k}") for k in range(DK)]
                    for k in range(DK):
                        nc.sync.dma_start(
                            xg[k][:].rearrange("p (r t) -> p r t", r=2),
                            xg_view[k][:, 2 * c:2 * c + 2, :].bitcast(DTR))
                    for m in range(4):
                        acc = psb.tile([128, 512], DT, tag="acc", name=f"acc{l}{c}{m}")
                        for k in range(DK):
                            nc.tensor.matmul(
                                acc[:],
                                r(wq_sb[:, k * 512 + m * 128: k * 512 + (m + 1) * 128]),
                                r(xg[k][:]),
                                start=(k == 0), stop=(k == DK - 1))
                        cs = slice(512 * c, 512 * (c + 1))
                        if m == 3:
                            nc.scalar.activation(vT[:, cs], acc[:], AF.Copy)
                            continue
                        # copy raw q/k out of PSUM immediately (frees the acc
                        # bank so the PE matmul stream stays dense), then run
                        # the qk-norm chain on ACT/DVE off the critical path
                        raw = (qT[m] if m < HQ else kT)[:, cs]
                        nc.scalar.activation(raw, acc[:], AF.Copy)
                        sq = pbs.tile([128, 512], DTR, tag="sqb", name=f"sqb{l}{c}{m}")
                        nc.scalar.activation(sq[:], acc[:], AF.Square)
                        ssq = psb.tile([1, 512], DT, tag="ssqb", name=f"ssqb{l}{c}{m}")
                        nc.tensor.matmul(ssq[:], r(ones_col), r(sq[:]),
                                         start=True, stop=True)
                        sqv = pbs.tile([1, 512], DT, tag="sqvb", name=f"sqvb{l}{c}{m}")
                        nc.scalar.activation(sqv[:], ssq[:], AF.Sqrt,
                                             scale=1.0 / HD, bias=eps_sb[0:1, :])
                        rinf = pbs.tile([1, 512], DT, tag="rinf", name=f"rinf{l}{c}{m}")
                        nc.vector.reciprocal_approx_fast(rinf[:], sqv[:])
                        rin = pbs.tile([1, 512], DTR, tag="rinb", name=f"rinb{l}{c}{m}")
                        nc.scalar.activation(rin[:], rinf[:], AF.Copy)
                        bw_ps = psb.tile([128, 512], DT, tag="bw", name=f"bw{l}{c}{m}",
                                         bufs=1)
                        w_sb = qn_sb if m < 2 else kn_sb
                        nc.tensor.matmul(bw_ps[:], r(w_sb[:]), r(rin[:]),
                                         start=True, stop=True)
                        bw_sb = pbs.tile([128, 512], DT, tag="bwsb",
                                         name=f"bwsb{l}{c}{m}")
                        nc.scalar.activation(bw_sb[:], bw_ps[:], AF.Copy)
                        qn_t = pbs.tile([128, 512], DT, tag="qnt", name=f"qnt{l}{c}{m}")
                        nc.vector.tensor_tensor(qn_t[:], raw, bw_sb[:], ALU.mult)
                        # rope: out = q*cos + rot_half(q)*sin_signed.
                        # qswap = q rotated by 64 partitions (SBUF DMA moves
                        # across partitions; DVE cannot).  sin_sb carries the
                        # sign: rows 0-63 = -sin, 64-127 = +sin.
                        dst = raw
                        qsw = pbs.tile([128, 512], DT, tag="qsw", name=f"qsw{l}{c}{m}")
                        nc.sync.dma_start(qsw[0:64, :], qn_t[64:128, :])
                        nc.sync.dma_start(qsw[64:128, :], qn_t[0:64, :])
                        nc.vector.tensor_tensor(dst, qn_t[:], cos_sb[:, cs], ALU.mult)
                        ru = pbs.tile([128, 512], DT, tag="ru", name=f"ru{l}{c}{m}")
                        nc.vector.tensor_tensor(ru[:], qsw[:], sin_sb[:, cs], ALU.mult)
                        nc.vector.tensor_tensor(dst, dst, ru[:], ALU.add)

                # transposes: v -> natural (PV lhsT + v_out).  k transposes
                # are deferred into the next layer's AllGather window.
                for i in range(16):
                    isl = slice(128 * i, 128 * (i + 1))
                    tpv = psb.tile([128, 128], DTR, tag="tp", name=f"tpv{l}{i}")
                    nc.tensor.transpose(tpv[:], vT[:, isl], id_sb[:])
                    nc.scalar.activation(vnat[:, isl], tpv[:], AF.Copy)
                    nc.sync.dma_start(v_out[l][isl, :].bitcast(DTR), vnat[:, isl])
                deferred_k.append((l, kT))
                psb_ctx.close()

                # ======== Phase C: attention (causal) ========
                with tc.tile_pool(name=f"pC{l}", bufs=3) as pc, \
                     tc.tile_pool(name=f"psC{l}", bufs=2, space="PSUM") as psc:
                    for hq in range(HQ):
                        for c in range(4):
                            cs = slice(512 * c, 512 * (c + 1))
                            pv = psc.tile([128, 512], DT, tag="pv",
                                          name=f"pv{l}{hq}{c}")
                            rs = psc.tile([1, 512], DT, tag="rs", name=f"rs{l}{hq}{c}")
                            nlast = 4 * c + 3
                            for i in range(4 * c + 4):
                                s_ps = psc.tile([128, 512], DT, tag="s",
                                                name=f"s{l}{hq}{c}{i}")
                                nc.tensor.matmul(
                                    s_ps[:], r(kT[:, 128 * i:128 * (i + 1)]),
                                    r(qT[hq][:, cs]), start=True, stop=True)
                                if i >= 4 * c:
                                    off = 128 * i - 512 * c
                                    nc.vector.tensor_tensor(
                                        s_ps[:], s_ps[:],
                                        mask_sb[:, 384 - off: 896 - off], ALU.add)
                                p_sb = pc.tile([128, 512], DTR, tag="p",
                                               name=f"p{l}{hq}{c}{i}")
                                nc.scalar.activation(p_sb[:], s_ps[:], AF.Exp,
                                                     scale=SCALE)
                                nc.tensor.matmul(
                                    pv[:], r(vnat[:, 128 * i:128 * (i + 1)]),
                                    r(p_sb[:]), start=(i == 0), stop=(i == nlast))
                                nc.tensor.matmul(
                                    rs[:], r(ones_col), r(p_sb[:]),
                                    start=(i == 0), stop=(i == nlast))
                            rr = pc.tile([1, 512], DT, tag="rr", name=f"rr{l}{hq}{c}")
                            nc.vector.reciprocal_approx_fast(rr[:], rs[:])
                            bcs = pc.tile([128, 512], DT, tag="bcs",
                                          name=f"bcs{l}{hq}{c}")
                            nc.gpsimd.partition_broadcast(bcs[:], rr[:])
                            ao = pc.tile([128, 512], DTR, tag="ao", name=f"ao{l}{hq}{c}")
                            nc.vector.tensor_tensor(ao[:], pv[:], bcs[:], ALU.mult)
                            for sub in range(2):
                                r0 = 128 * (2 * c + sub)
                                nc.sync.dma_start(
                                    a2a_in[hq][r0:r0 + 128, :],
                                    ao[:, 256 * sub:256 * (sub + 1)])
                        # per-head AllToAll: head 0's exchange overlaps head 1's
                        # attention; o-proj starts on the first half
                        nc.gpsimd.collective_compute(
                            "AllToAll", ALU.bypass,
                            replica_groups=[list(range(NC))],
                            ins=[a2a_in[hq].opt()], outs=[a2a_out[hq].opt()])

            # ======== Phase D: o-proj + residual ========
            with tc.tile_pool(name=f"pD{l}", bufs=1) as pd, \
                 tc.tile_pool(name=f"pDw{l}", bufs=2) as pdw, \
                 tc.tile_pool(name=f"psD{l}", bufs=2, space="PSUM") as psd:
                ax = []
                for hx in range(HQ):
                    ax_h = pd.tile([128, NC * TC], DTR, name=f"ax{l}{hx}")
                    nc.sync.dma_start(
                        ax_h[:].rearrange("p (k t) -> p k t", k=NC),
                        a2a_out[hx].rearrange("(k p) t -> p k t", k=NC))
                    ax.append(ax_h)
                # accumulate even global heads (from a2a #0) first so o-proj
                # starts as soon as the first exchange lands
                order = [(hx, rk) for hx in range(HQ) for rk in range(NC)]
                for d in range(DK):
                    ow_sb = pdw.tile([128, 16 * 128], DTR, tag="ow", name=f"ow{l}{d}")
                    nc.sync.dma_start(ow_sb[:], owt[l, d].bitcast(DTR))
                    op_ps = psd.tile([128, TC], DT, tag="op", name=f"op{l}{d}")
                    for j, (hx, rk) in enumerate(order):
                        g = 2 * rk + hx
                        nc.tensor.matmul(
                            op_ps[:], r(ow_sb[:, g * 128:(g + 1) * 128]),
                            r(ax[hx][:, rk * TC:(rk + 1) * TC]),
                            start=(j == 0), stop=(j == 15))
                    dsl = slice(d * TC, (d + 1) * TC)
                    nc.vector.tensor_tensor(h_sb[:, dsl], h_sb[:, dsl], op_ps[:],
                                            ALU.add)

            # ======== Phase E: MLP ========
            with tc.tile_pool(name=f"pE{l}", bufs=1) as pe, \
                 tc.tile_pool(name=f"pEs{l}", bufs=2) as pes, \
                 tc.tile_pool(name=f"pEw{l}", bufs=3) as pew, \
                 tc.tile_pool(name=f"psE{l}", bufs=2, space="PSUM") as pse:
                rinv2 = rms_rinv(pes, pse, h_sb, DK, float(D), f"y{l}")
                bwy = pes.tile([128, TC], DT, tag="bwy", name=f"bwy{l}")
                nc.gpsimd.partition_broadcast(bwy[:], rinv2[:])
                y_sb = pe.tile([128, DK * TC], DTR, name=f"y_sb{l}")
                for d in range(DK):
                    dsl = slice(d * TC, (d + 1) * TC)
                    nc.vector.tensor_tensor(y_sb[:, dsl], h_sb[:, dsl], bwy[:],
                                            ALU.mult)
                gu_sb = pe.tile([128, FK * TC], DTR, name=f"gu_sb{l}")
                for f in range(FK):
                    gw = pew.tile([128, DK * 128], DTR, tag="gw", name=f"gw{l}{f}",
                                  bufs=4)
                    uw = pew.tile([128, DK * 128], DTR, tag="uw", name=f"uw{l}{f}",
                                  bufs=4)
                    nc.sync.dma_start(gw[:], gatewt[l, f].bitcast(DTR))
                    nc.sync.dma_start(uw[:], upwt[l, f].bitcast(DTR))
                    g_ps = pse.tile([128, TC], DT, tag="g", name=f"g{l}{f}")
                    u_ps = pse.tile([128, TC], DT, tag="u", name=f"u{l}{f}")
                    for k in range(DK):
                        ksl = slice(k * 128, (k + 1) * 128)
                        ysl = slice(k * TC, (k + 1) * TC)
                        nc.tensor.matmul(g_ps[:], r(gw[:, ksl]), r(y_sb[:, ysl]),
                                         start=(k == 0), stop=(k == DK - 1))
                        nc.tensor.matmul(u_ps[:], r(uw[:, ksl]), r(y_sb[:, ysl]),
                                         start=(k == 0), stop=(k == DK - 1))
                    sg = pes.tile([128, TC], DT, tag="sg", name=f"sg{l}{f}")
                    nc.scalar.activation(sg[:], g_ps[:], AF.Sigmoid)
                    gs = pes.tile([128, TC], DT, tag="gs", name=f"gs{l}{f}")
                    nc.vector.tensor_tensor(gs[:], sg[:], g_ps[:], ALU.mult)
                    fsl = slice(f * TC, (f + 1) * TC)
                    nc.vector.tensor_tensor(gu_sb[:, fsl], gs[:], u_ps[:], ALU.mult)
                for d in range(DK):
                    dw = pew.tile([128, FK * 128], DTR, tag="dw", name=f"dw{l}{d}",
                                  bufs=2)
                    nc.sync.dma_start(dw[:], downwt[l, d].bitcast(DTR))
                    dp = pse.tile([128, TC], DT, tag="dp", name=f"dp{l}{d}")
                    for f in range(FK):
                        nc.tensor.matmul(
                            dp[:], r(dw[:, f * 128:(f + 1) * 128]),
                            r(gu_sb[:, f * TC:(f + 1) * TC]),
                            start=(f == 0), stop=(f == FK - 1))
                    dsl = slice(d * TC, (d + 1) * TC)
                    nc.vector.tensor_tensor(h_sb[:, dsl], h_sb[:, dsl], dp[:],
                                            ALU.add)

        # ======== Final norm + transpose + store ========
        with tc.tile_pool(name="pF", bufs=2) as pf, \
             tc.tile_pool(name="psF", bufs=2, space="PSUM") as psf:
            emit_k_out(pf, psf)
            rinvf = rms_rinv(pf, psf, h_sb, DK, float(D), "f")
            rinfd = pf.tile([1, TC], DTR, name="rinfd")
            nc.scalar.activation(rinfd[:], rinvf[:], AF.Copy)
            for d in range(DK):
                bcf = psf.tile([128, TC], DT, tag="bc", name=f"bcf{d}", bufs=1)
                nc.tensor.matmul(bcf[:], r(nw_sb[:, d * 128:(d + 1) * 128]),
                                 r(rinfd[:]), start=True, stop=True)
                hs_sb = pf.tile([128, TC], DTR, tag="hs", name=f"hs{d}")
                nc.vector.tensor_tensor(hs_sb[:], h_sb[:, d * TC:(d + 1) * TC],
                                        bcf[:], ALU.mult)
                for s in range(2):
                    tpf = psf.tile([128, 128], DTR, tag="tp", name=f"tpf{d}{s}")
                    nc.tensor.transpose(tpf[:], hs_sb[:, 128 * s:128 * (s + 1)],
                                        id_sb[:])
                    tfs = pf.tile([128, 128], DT, tag="tfs", name=f"tfs{d}{s}")
                    nc.scalar.activation(tfs[:], tpf[:], AF.Copy)
                    nc.sync.dma_start(
                        hs_out[128 * s:128 * (s + 1), 128 * d:128 * (d + 1)],
                        tfs[:])
    nc.compile()
    return nc


# ---------------- host side ----------------

def _rope_tables():
    inv_freq = 1.0 / (ROPE_THETA ** (np.arange(0, HD, 2, dtype=np.float64) / HD))
    pos = np.arange(T, dtype=np.float64)
    freqs = pos[:, None] * inv_freq[None, :]
    cos_h = np.cos(freqs).T.astype(np.float32)           # (64, T)
    sin_h = np.sin(freqs).T.astype(np.float32)
    cos128 = np.concatenate([cos_h, cos_h], axis=0)      # duplicated halves
    sin128 = np.concatenate([-sin_h, sin_h], axis=0)     # sign folded in
    return np.ascontiguousarray(cos128), np.ascontiguousarray(sin128)


def _mask_buffer():
    u = np.arange(896)[None, :]
    p = np.arange(128)[:, None]
    return np.where(u >= p + 384, 0.0, NEG).astype(np.float32)


def host_prep(inputs):
    f = lambda a: np.ascontiguousarray(np.asarray(a, np.float32))
    emb = f(inputs["inputs_embeds"])[0]
    ln1, ln2 = f(inputs["ln1_w"]), f(inputs["ln2_w"])
    qw, kw, vw = f(inputs["q_w"]), f(inputs["k_w"]), f(inputs["v_w"])
    qn, kn = f(inputs["qn_w"]), f(inputs["kn_w"])
    ow = f(inputs["o_w"])
    gw, uw, dw = f(inputs["gate_w"]), f(inputs["up_w"]), f(inputs["down_w"])
    nw = f(inputs["norm_w"])

    cosT, sinT = _rope_tables()
    maskB = _mask_buffer()
    ident = np.eye(128, dtype=np.float32)
    ones = np.ones((128, 128), np.float32)

    owT = ow.transpose(0, 2, 1)
    owt = np.ascontiguousarray(
        owT.reshape(L, 16, 128, DK, 128).transpose(0, 3, 2, 1, 4)
           .reshape(L, DK, 128, 16 * 128))

    def guw_tile(w):
        wf = (w * ln2[:, None, :]).transpose(0, 2, 1)
        return np.ascontiguousarray(
            wf.reshape(L, DK, 128, FK, 128).transpose(0, 3, 2, 1, 4)
              .reshape(L, FK, 128, DK * 128))
    gatewt, upwt = guw_tile(gw), guw_tile(uw)

    dwT = dw.transpose(0, 2, 1)
    downwt = np.ascontiguousarray(
        dwT.reshape(L, FK, 128, DK, 128).transpose(0, 3, 2, 1, 4)
           .reshape(L, DK, 128, FK * 128))

    shared = dict(owt=owt, gatewt=gatewt, upwt=upwt, downwt=downwt,
                  qnw=np.ascontiguousarray(qn[:, None, :]),
                  knw=np.ascontiguousarray(kn[:, None, :]),
                  normw=np.ascontiguousarray(nw[None, :]),
                  cost=cosT, sint=sinT, maskb=maskB, ident=ident, ones=ones)

    in_maps = []
    for c in range(NC):
        qs = qw[:, 256 * c:256 * (c + 1), :] * ln1[:, None, :]
        ks = kw[:, 128 * c:128 * (c + 1), :] * ln1[:, None, :]
        vs = vw[:, 128 * c:128 * (c + 1), :] * ln1[:, None, :]
        wT = np.concatenate([qs, ks, vs], axis=1).transpose(0, 2, 1)  # (L, D, 512)
        qkvw = np.ascontiguousarray(
            wT.reshape(L, DK, 128, 512).transpose(0, 2, 1, 3)
              .reshape(L, 128, DK * 512))
        embt = np.ascontiguousarray(
            emb.T[:, TC * c:TC * (c + 1)].reshape(DK, 128, TC))
        in_maps.append(dict(embt=embt, qkvw=qkvw, **shared))
    return in_maps


def assemble(results):
    hs = np.concatenate([res["hs_out"] for res in results], axis=0)[None]
    keys = np.stack([np.stack([res["k_out"][l] for res in results])
                     for l in range(L)])[:, None]
    vals = np.stack([np.stack([res["v_out"][l] for res in results])
                     for l in range(L)])[:, None]
    return (np.ascontiguousarray(hs, dtype=np.float32),
            np.ascontiguousarray(keys, dtype=np.float32),
            np.ascontiguousarray(vals, dtype=np.float32))


_CACHE = {}


def kernel(**inputs):
    from concourse.bass_utils import run_bass_kernel_spmd
    if "nc" not in _CACHE:
        _CACHE["nc"] = build_program()
    in_maps = host_prep(inputs)
    res = run_bass_kernel_spmd(_CACHE["nc"], in_maps, core_ids=list(range(NC)))
    return assemble(res.results)
